# revision 1
# baseline (speedup 1.0000x reference)
"""Trainium2 Bass kernel for nn_Align: batched quaternion->rotmat + rigid transform.

reference math (per structure j of 64):
    q = (1, b, c, d) / sqrt(s),  s = 1 + b^2 + c^2 + d^2
    R = rotmat(q)                       # 3x3
    out[j] = pred[j] @ R + t[j]         # [91,3] @ [3,3] + [3]

Sharding: data-parallel over the 8 NeuronCores, 8 structures per core.

Per-core layout: partitions = (structure j:8, point-group g:13) = 104,
free dim = (point-in-group q:7, coord m:3) = 21.  R = N * (2/s) - I with
numerators N assembled from one broadcast-AP product op over the packed
row tail rc = [1 b c d b c]:  P[:, a+4b] = rc_a * rc_{a+b} gives
[1 bb cc dd | b bc cd db], so sum(P[0:4]) = s and the off-diagonal
products sit contiguously.  The transform runs as 9 fused
per-partition-scalar multiply-adds (3 per output coordinate).

Raw Bass (no Tile: this walrus build encodes at most one sync-wait per
compute instruction).  Every DVE RAW dep is semaphore-synced (streaming
same-engine RAW is not safe on HW), and the kernel clears its semaphores
then all-engine-barriers before use (sems persist across NEFF runs).
"""

import numpy as np

NCORES = 8
J = 8         # structures per core
G = 13        # point groups per structure
Q = 7         # points per group  (G*Q = 91)
PARTS = J * G  # 104 partitions

# R-tile column layout: [0:3]=diag(R00,R11,R22) [3:6]=plus(R10,R21,R02)
# [6:9]=minus(R01,R12,R20).  Columns holding (R[0,n], R[1,n], R[2,n]):
CHANNEL_COLS = {0: (0, 3, 8), 1: (6, 1, 4), 2: (5, 7, 2)}

_cache = {}


def _build_nc():
    import dataclasses

    import concourse.bass as bass
    import concourse.mybir as mybir

    f32 = mybir.dt.float32
    Alu = mybir.AluOpType

    nc = bass.Bass()
    # host-packed per (structure, point-group) row (30 floats):
    # [ 21 pred floats (7 points x 3 coords) | 1 b c d b c t0 t1 t2 ]
    packed = nc.dram_tensor("packed", [PARTS, 30], f32, kind="ExternalInput")
    out = nc.dram_tensor("out", [J, 91, 3], f32, kind="ExternalOutput")

    with (
        nc.sbuf_tensor([PARTS, 30], f32) as PK_t,
        nc.sbuf_tensor([PARTS, 8], f32) as P_t,
        nc.sbuf_tensor([PARTS, 9], f32) as R_t,
        nc.sbuf_tensor([PARTS, 1], f32) as S2_t,
        nc.sbuf_tensor([PARTS, 1], f32) as INV_t,
        nc.sbuf_tensor([PARTS, 6 * Q], f32) as ACC_t,
        nc.sbuf_tensor([PARTS, 21], f32) as O_t,
        nc.semaphore("dma_in") as dma_in_sem,
        nc.semaphore("v") as v_sem,
        nc.semaphore("dve_done") as dve_sem,
        nc.semaphore("dma_out") as dma_out_sem,
        nc.Block() as block,
    ):
        PK = PK_t[:, :]
        P = P_t[:, :]
        R = R_t[:, :]
        S2 = S2_t[:, :]
        INV = INV_t[:, :]
        O = O_t[:, :]
        ACC = [ACC_t[:, i * Q:(i + 1) * Q] for i in range(6)]
        RT = PK[:, 21:30]   # [1 b c d b c t0 t1 t2]

        def _pseudo_barrier(eng):
            # NRT expands this to a real all-engine barrier on runtime
            # semaphores outside the kernel sem range — stale-state proof.
            eng.isa(
                nc.isa.Opcode.NEURON_ISA_TPB_OPCODE_PSEUDO_SYNC_BARRIER,
                {},
                struct_name="NEURON_ISA_TPB_UNKNOWN_STRUCT",
                verify=False,
            )

        @block.gpsimd
        def _(gpsimd):
            # Stale-semaphore preamble: semaphores are NOT reset between NEFF
            # executions, and waits here use absolute values.  Clear every sem
            # this kernel waits on or increments, THEN barrier — without the
            # barrier an engine can pass its first wait on a stale value
            # before the clear lands (observed as a HW deadlock).  The Block
            # exit barrier's event sems (nc.barrier_sems) are self-managed
            # and were never cleared by the framework preamble either.
            nums = sorted(x.num for x in (dma_in_sem, v_sem, dve_sem, dma_out_sem))
            assert nums[-1] - nums[0] == 3, nums
            r = range(nums[0], nums[-1] + 1)
            gpsimd.dma_reset(r)
            gpsimd.sem_clear(r)
            _pseudo_barrier(gpsimd)

        @block.scalar
        def _(scalar):
            _pseudo_barrier(scalar)

        @block.tensor
        def _(tensor):
            _pseudo_barrier(tensor)

        @block.sync
        def _(sync):
            _pseudo_barrier(sync)
            sync.dma_start(out=PK, in_=packed[:, :]).then_inc(dma_in_sem, 16)
            sync.wait_ge(dve_sem, 1)
            sync.dma_start(
                out=out[:, :, :].rearrange("j (g q) m -> (j g) (q m)", g=G),
                in_=O,
            ).then_inc(dma_out_sem, 16)
            sync.wait_ge(dma_out_sem, 16)

        @block.vector
        def _(vector):
            _pseudo_barrier(vector)
            vector.wait_ge(dma_in_sem, 16)

            # DVE streaming RAW is not safe without sem sync (HW-verified):
            # every op bumps v_sem; consumers wait on the cumulative count.
            def op(k, *args, **kw):
                return getattr(vector, k)(*args, **kw).then_inc(v_sem, 1)

            # ---- rotation matrix ----
            # P[:, a+4b] = u_a * u_{a+b} over rc = RT[0:6] = [1 b c d b c],
            # b in {0,1}, a in {0,3}:
            #   b=0 -> [1 bb cc dd]   (cols 0:4; sum = s)
            #   b=1 -> [b bc cd db]   (cols 4:8; bc,cd,db at 5:8)
            u_ap = RT[:, 0:4].unsqueeze(1).broadcast_to([PARTS, 2, 4])
            v_base = RT[:, 0:4].unsqueeze(1).broadcast_to([PARTS, 2, 4])
            pairs = [list(p) for p in v_base.ap]
            pairs[1][0] = 1  # dims [partition, b, a]; b-step 1 elem -> u_{a+b}
            v_ap = dataclasses.replace(v_base, ap=pairs)
            p_out = P.rearrange("p (b a) -> p b a", b=2)
            op("tensor_tensor", out=p_out, in0=u_ap, in1=v_ap, op=Alu.mult)  # 1
            vector.wait_ge(v_sem, 1)
            op("reduce_sum", out=S2, in_=P[:, 0:4],                          # 2  s
               axis=mybir.AxisListType.X)
            vector.wait_ge(v_sem, 2)
            op("reciprocal", out=INV, in_=S2)                                # 3  1/s
            # numerators: diag = P[1:4]+1;  plus/minus = [bc,cd,db] -+ [d,b,c]
            op("tensor_scalar", out=R[:, 0:3], in0=P[:, 1:4], scalar1=1.0,   # 4
               scalar2=None, op0=Alu.add)
            op("tensor_tensor", out=R[:, 3:6], in0=P[:, 5:8],                # 5
               in1=RT[:, 3:6], op=Alu.add)
            op("tensor_tensor", out=R[:, 6:9], in0=P[:, 5:8],                # 6
               in1=RT[:, 3:6], op=Alu.subtract)
            vector.wait_ge(v_sem, 6)
            op("tensor_scalar", out=R, in0=R, scalar1=INV, scalar2=2.0,      # 7
               op0=Alu.mult, op1=Alu.mult)                                   #   R=num*2/s
            vector.wait_ge(v_sem, 7)
            op("tensor_scalar", out=R[:, 0:3], in0=R[:, 0:3], scalar1=-1.0,  # 8
               scalar2=None, op0=Alu.add)                                    #   diag -1

            # ---- transform (channel-interleaved) ----
            xm = PK[:, 0:21].rearrange("p (q m) -> p m q", m=3)
            om = O.rearrange("p (q m) -> p m q", m=3)
            a0 = [ACC[2 * n][:, :] for n in range(3)]
            a1 = [ACC[2 * n + 1][:, :] for n in range(3)]
            vector.wait_ge(v_sem, 8)
            for n in range(3):        # 9,10,11:  I1_n = X0*R[0,n] + t_n
                c0 = CHANNEL_COLS[n][0]
                op("tensor_scalar", out=a0[n], in0=xm[:, 0, :],
                   scalar1=R[:, c0:c0 + 1], scalar2=RT[:, 6 + n:7 + n],
                   op0=Alu.mult, op1=Alu.add)
            for n in range(3):        # 12,13,14:  I2_n = X1*R[1,n] + I1_n
                vector.wait_ge(v_sem, 9 + n)
                c1 = CHANNEL_COLS[n][1]
                op("scalar_tensor_tensor", out=a1[n], in0=xm[:, 1, :],
                   scalar=R[:, c1:c1 + 1], in1=a0[n],
                   op0=Alu.mult, op1=Alu.add)
            for n in range(3):        # 15,16,17:  out_n = X2*R[2,n] + I2_n
                vector.wait_ge(v_sem, 12 + n)
                c2 = CHANNEL_COLS[n][2]
                ins = vector.scalar_tensor_tensor(
                    out=om[:, n, :], in0=xm[:, 2, :],
                    scalar=R[:, c2:c2 + 1], in1=a1[n],
                    op0=Alu.mult, op1=Alu.add,
                )
                if n < 2:
                    ins.then_inc(v_sem, 1)
                else:
                    ins.then_inc(dve_sem, 1)

    return nc


def get_nc():
    if "nc" not in _cache:
        _cache["nc"] = _build_nc()
    return _cache["nc"]


def shard_inputs(pred_coor, r_vector, t_vector):
    # packed per (structure, group) row: [21 pred | 1 b c d b c | t0 t1 t2]
    n = pred_coor.shape[0]
    pk = np.empty((n, G, 30), dtype=np.float32)
    pk[:, :, 0:21] = pred_coor.reshape(n, G, 21)
    pk[:, :, 21] = 1.0
    pk[:, :, 22:25] = r_vector[:, None, :]
    pk[:, :, 25:27] = r_vector[:, None, 0:2]
    pk[:, :, 27:30] = t_vector[:, None, :]
    pk = pk.reshape(n * G, 30)
    return [
        {"packed": np.ascontiguousarray(pk[c * PARTS : (c + 1) * PARTS])}
        for c in range(NCORES)
    ]


def run(pred_coor, r_vector, t_vector, trace=False):
    from concourse.bass_utils import run_bass_kernel_spmd

    nc = get_nc()
    in_maps = shard_inputs(pred_coor, r_vector, t_vector)
    res = run_bass_kernel_spmd(nc, in_maps, list(range(NCORES)), trace=trace)
    full = np.concatenate([res.results[c]["out"] for c in range(NCORES)], axis=0)
    return full, res


def kernel(pred_coor, r_vector, t_vector):
    pred_coor = np.asarray(pred_coor, dtype=np.float32)
    r_vector = np.asarray(r_vector, dtype=np.float32)
    t_vector = np.asarray(t_vector, dtype=np.float32)
    full, _ = run(pred_coor, r_vector, t_vector, trace=False)
    return full



# revision 2
# speedup vs baseline: 1.0614x; 1.0614x over previous
"""Trainium2 Bass kernel for nn_Align v2: quaternion->rotmat + rigid transform.

Math (per structure j):  out = X @ R + t,  R = (2/s) M - I  with
  M[m][n] = u_{m+1} u_{n+1} + Lin[m][n],  u = (1,b,c,d),  s = sum u_i^2,
  Lin = [[1,-d,c],[d,1,-b],[-c,b,1]].
Rewritten as  out = (1/s) * (Xd @ M) + (t - X)  with Xd = 2X host-packed,
so the division folds into the final fused multiply-add (8 DVE ops total):
  1. P16 = outer(u,u)            [104,16]
  2. TX  = -0.5*Xd + t           [104,21]   (only needs DMA data)
  3. S   = reduce(diag P16) = s  [104,1]
  4. M   = P16[1:,1:] + Lin      [104,9]
  5. INV = 1/s                   [104,1]
  6. U   = Xd (x) M  broadcast   [104,63]   U[q,n,m] = Xd[q,m]*M[m,n]
  7. Y   = reduce_m U            [104,21]
  8. O   = INV*Y + TX            [104,21]

Output DMA: SWDGE kv_writeback descriptors are PREPARED on the Pool engine
during the input DMA (994ns fixed cost hidden), then TRIGGERED after the
last DVE op — post-compute tail is seq+transfer+sem (~1.0us) instead of a
full HWDGE chain (~2.3us).  kv_writeback with batch=1, d_head=128, dho=1,
ncn=n_ctx=21, ctx_idx=0 degenerates to dst_row[p][0:21] = src[p][0:21] for
all 128 partitions; the out DRAM tensor is declared [128,21] and the host
keeps rows 0:104.

Raw Bass; every DVE RAW dep is semaphore-synced, and the kernel clears its
semaphores then all-engine-barriers before use (sems persist across runs).
"""

import numpy as np

NCORES = 8
J = 8          # structures per core
G = 13         # point groups per structure
Q = 7          # points per group  (G*Q = 91)
PARTS = J * G  # 104 partitions
ROW = 37       # packed row: [Xd(21) | u4(4) | Lin(9) | t(3)]

_cache = {}


def _build_nc():
    import dataclasses

    import concourse.bass as bass
    import concourse.mybir as mybir
    from concourse import library_config

    f32 = mybir.dt.float32
    Alu = mybir.AluOpType

    def restride(ap, dim, stride, count=None):
        pairs = [list(p) for p in ap.ap]
        pairs[dim][0] = stride
        if count is not None:
            pairs[dim][1] = count
        return dataclasses.replace(ap, ap=pairs)

    nc = bass.Bass()
    packed = nc.dram_tensor("packed", [PARTS, ROW], f32, kind="ExternalInput")
    out = nc.dram_tensor("out", [128, 21], f32, kind="ExternalOutput")

    with (
        nc.sbuf_tensor([PARTS, ROW], f32) as PK_t,
        nc.sbuf_tensor([PARTS, 16], f32) as P16_t,
        nc.sbuf_tensor([PARTS, 21], f32) as TX_t,
        nc.sbuf_tensor([PARTS, 1], f32) as S_t,
        nc.sbuf_tensor([PARTS, 1], f32) as INV_t,
        nc.sbuf_tensor([PARTS, 9], f32) as M_t,
        nc.sbuf_tensor([PARTS, 63], f32) as U_t,
        nc.sbuf_tensor([PARTS, 21], f32) as Y_t,
        nc.sbuf_tensor([128, 21], f32) as O_t,
        nc.semaphore("dma_in") as dma_in_sem,
        nc.semaphore("v") as v_sem,
        nc.semaphore("dve_done") as dve_sem,
        nc.semaphore("prep") as prep_sem,
        nc.semaphore("dma_out") as dma_out_sem,
        nc.Block() as block,
    ):
        PK = PK_t[:, :]
        Xd = PK[:, 0:21]                    # 2*pred, (q,m) row-major
        U4 = PK[:, 21:25]                   # [1,b,c,d]
        LIN = PK[:, 25:34].rearrange("p (m n) -> p m n", m=3)
        T3 = PK[:, 34:37]

        def _pseudo_barrier(eng):
            # NRT expands this to a real all-engine barrier on runtime
            # semaphores outside the kernel sem range — stale-state proof.
            eng.isa(
                nc.isa.Opcode.NEURON_ISA_TPB_OPCODE_PSEUDO_SYNC_BARRIER,
                {},
                struct_name="NEURON_ISA_TPB_UNKNOWN_STRUCT",
                verify=False,
            )

        @block.gpsimd
        def _(gpsimd):
            # Stale-semaphore preamble: sems are NOT reset between NEFF
            # executions and waits use absolute values.  Clear every sem this
            # kernel touches, THEN barrier (see v1 docstring for the deadlock
            # this prevents).
            sems = (dma_in_sem, v_sem, dve_sem, prep_sem, dma_out_sem)
            nums = sorted(x.num for x in sems)
            assert nums[-1] - nums[0] == len(sems) - 1, nums
            r = range(nums[0], nums[-1] + 1)
            gpsimd.dma_reset(r)
            gpsimd.sem_clear(r)
            _pseudo_barrier(gpsimd)

            # kv_writeback lives in the attn gpsimd library.
            gpsimd.load_library(library_config.attn)
            # Partitions 104..127 of O are never computed; zero them once so
            # the writeback ships defined bytes (host discards those rows).
            gpsimd.memset(O_t[104:128, :], 0.0)

            # Prepare the output descriptors now (SWDGE gen ~1us, hidden
            # behind the input DMA + compute); source data is read at
            # trigger time, not prep time.
            in4 = dataclasses.replace(
                O_t[:, :], ap=[[21, 128], [21, 1], [21, 1], [1, 21]]
            )
            out4 = dataclasses.replace(
                out[:, :], ap=[[2688, 1], [21, 128], [21, 1], [1, 21]]
            )
            idxs = nc.const_aps.aps[(f32, 0.0)].bitcast(mybir.dt.int32)
            gpsimd.kv_writeback(
                out_ap=out4, in_ap=in4, ctx_idxs_ap=idxs,
                prepare_only=True, sem=dma_out_sem,
            ).then_inc(prep_sem, 1)
            gpsimd.wait_ge(prep_sem, 1)
            gpsimd.wait_ge(dve_sem, 1)
            gpsimd.trigger_dma(count=1)
            gpsimd.wait_ge(dma_out_sem, 16)

        @block.scalar
        def _(scalar):
            _pseudo_barrier(scalar)

        @block.tensor
        def _(tensor):
            _pseudo_barrier(tensor)

        @block.sync
        def _(sync):
            _pseudo_barrier(sync)
            sync.dma_start(out=PK, in_=packed[:, :]).then_inc(dma_in_sem, 16)

        @block.vector
        def _(vector):
            _pseudo_barrier(vector)
            vector.wait_ge(dma_in_sem, 16)

            # DVE streaming RAW is not safe without sem sync (HW-verified):
            # every op bumps v_sem; consumers wait on the producer's count.
            def op(k, *args, **kw):
                return getattr(vector, k)(*args, **kw).then_inc(v_sem, 1)

            # 1: P16[i,j] = u_i * u_j
            u_i = U4.unsqueeze(2).broadcast_to([PARTS, 4, 4])
            u_j = U4.unsqueeze(1).broadcast_to([PARTS, 4, 4])
            p16 = P16_t[:, :].rearrange("p (i j) -> p i j", i=4)
            op("tensor_tensor", out=p16, in0=u_i, in1=u_j, op=Alu.mult)
            # 2: TX = -0.5*Xd + t   (dep: DMA only)
            t_b = T3.unsqueeze(1).broadcast_to([PARTS, Q, 3])
            xqm = Xd.rearrange("p (q m) -> p q m", q=Q)
            txq = TX_t[:, :].rearrange("p (q m) -> p q m", q=Q)
            op("scalar_tensor_tensor", out=txq, in0=xqm, scalar=-0.5,
               in1=t_b, op0=Alu.mult, op1=Alu.add)
            # 3: S = sum(diag P16) = s        (dep 1)
            vector.wait_ge(v_sem, 1)
            diag = restride(P16_t[:, 0:4], 1, 5)
            op("reduce_sum", out=S_t[:, :], in_=diag, axis=mybir.AxisListType.X)
            # 4: M = P16[1:,1:] + Lin         (dep 1)
            pmn = dataclasses.replace(
                P16_t[:, 5:6], ap=[list(P16_t[:, 5:6].ap[0]), [4, 3], [1, 3]]
            )
            m_mn = M_t[:, :].rearrange("p (m n) -> p m n", m=3)
            op("tensor_tensor", out=m_mn, in0=pmn, in1=LIN, op=Alu.add)
            # 5: INV = 1/s                    (dep 3)
            vector.wait_ge(v_sem, 3)
            op("reciprocal", out=INV_t[:, :], in_=S_t[:, :])
            # 6: U[q,n,m] = Xd[q,m] * M[m,n]  (dep 4)
            vector.wait_ge(v_sem, 4)
            x_qnm = xqm.unsqueeze(2).broadcast_to([PARTS, Q, 3, 3])
            m_qnm = (
                M_t[:, :].rearrange("p (m n) -> p n m", m=3)
                .unsqueeze(1).broadcast_to([PARTS, Q, 3, 3])
            )
            u_out = U_t[:, :].rearrange("p (q n m) -> p q n m", q=Q, n=3)
            op("tensor_tensor", out=u_out, in0=x_qnm, in1=m_qnm, op=Alu.mult)
            # 7: Y = reduce_m U               (dep 6)
            vector.wait_ge(v_sem, 6)
            u_tm = U_t[:, :].rearrange("p (t m) -> p t m", m=3)
            op("reduce_sum", out=Y_t[:, :], in_=u_tm, axis=mybir.AxisListType.X)
            # 8: O = INV*Y + TX               (deps 2,5,7)
            vector.wait_ge(v_sem, 7)
            vector.scalar_tensor_tensor(
                out=O_t[0:PARTS, :], in0=Y_t[:, :], scalar=INV_t[:, 0:1],
                in1=TX_t[:, :], op0=Alu.mult, op1=Alu.add,
            ).then_inc(dve_sem, 1)

    return nc


def get_nc():
    if "nc" not in _cache:
        nc = _build_nc()
        # Raw Bass skips Bacc's codegen_inst_isa_subclasses pass; without it
        # extended insts (trigger_dma, load_library) serialize with empty
        # .instr bytes and walrus codegen fails with "ISA wrong length".
        import concourse.mybir as mybir

        mybir.codegen_inst_isa_subclasses(nc)
        _cache["nc"] = nc
    return _cache["nc"]


def shard_inputs(pred_coor, r_vector, t_vector):
    # packed per (structure, group) row: [2*pred(21) | 1 b c d | Lin(9) | t(3)]
    n = pred_coor.shape[0]
    b, c, d = r_vector[:, 0], r_vector[:, 1], r_vector[:, 2]
    one = np.ones_like(b)
    lin = np.stack([one, -d, c, d, one, -b, -c, b, one], axis=-1)  # [n, 9]
    pk = np.empty((n, G, ROW), dtype=np.float32)
    pk[:, :, 0:21] = 2.0 * pred_coor.reshape(n, G, 21)
    pk[:, :, 21] = 1.0
    pk[:, :, 22:25] = r_vector[:, None, :]
    pk[:, :, 25:34] = lin[:, None, :]
    pk[:, :, 34:37] = t_vector[:, None, :]
    pk = pk.reshape(n * G, ROW)
    return [
        {"packed": np.ascontiguousarray(pk[k * PARTS : (k + 1) * PARTS])}
        for k in range(NCORES)
    ]


def run(pred_coor, r_vector, t_vector, trace=False):
    from concourse.bass_utils import run_bass_kernel_spmd

    nc = get_nc()
    in_maps = shard_inputs(pred_coor, r_vector, t_vector)
    res = run_bass_kernel_spmd(nc, in_maps, list(range(NCORES)), trace=trace)
    full = np.concatenate(
        [res.results[k]["out"][:PARTS].reshape(J, 91, 3) for k in range(NCORES)],
        axis=0,
    )
    return full, res


def kernel(pred_coor, r_vector, t_vector):
    pred_coor = np.asarray(pred_coor, dtype=np.float32)
    r_vector = np.asarray(r_vector, dtype=np.float32)
    t_vector = np.asarray(t_vector, dtype=np.float32)
    full, _ = run(pred_coor, r_vector, t_vector, trace=False)
    return full
